# revision 3
# baseline (speedup 1.0000x reference)
"""VQ codebook nearest-neighbor encode kernel for Trainium2 (8 NeuronCores).

Pipeline (matches the reference nn.Module):
  1. 3x3 SAME conv  [B,512,8,8] -> [B,256,8,8]   (9 shifted matmuls, PSUM accum)
  2. permute + linear on channel dim              (matmuls)
  3. nearest codeword: argmin_k |x - e_k|^2 = argmax_k (x.e_k - 0.5|e_k|^2)
     computed as fp32 matmuls + fused subtract + DVE max8/max_index.

Sharding: data-parallel over batch. Each of the 8 cores processes 32 images
(2048 positions) end-to-end with a full copy of the (small) weights and the
codebook. No collectives; host concatenates the 8 index shards.
"""

import sys

sys.path.insert(0, "/opt/trn_rl_repo")

import numpy as np

import concourse.bass as bass  # noqa: F401  (registers engines)
import concourse.tile as tile
from concourse import bacc, mybir
from concourse.bass_utils import run_bass_kernel_spmd

F32 = mybir.dt.float32
U32 = mybir.dt.uint32
I32 = mybir.dt.int32

B, CIN, COUT, H, W, K = 256, 512, 256, 8, 8, 8192
NCORES = 8
BL = B // NCORES          # images per core (32)
NPOS = BL * H * W         # positions per core (2048)
NPC = NPOS // 128         # pos-chunks per core (16)
NKC = K // 512            # codebook chunks (16)

_CACHE = {}


def build_kernel(n_iters: int = 1):
    nc = bacc.Bacc("TRN2", target_bir_lowering=False, debug=False,
                   num_devices=NCORES)
    lat_ap = nc.dram_tensor("lat", [BL, CIN, H, W], F32, kind="ExternalInput").ap()
    wt_ap = nc.dram_tensor("wt", [3, 3, CIN, COUT], F32, kind="ExternalInput").ap()
    cb_ap = nc.dram_tensor("cb", [COUT], F32, kind="ExternalInput").ap()
    lw_ap = nc.dram_tensor("lw", [COUT, COUT], F32, kind="ExternalInput").ap()  # lin_w.T: [cout_in, dout]
    lb_ap = nc.dram_tensor("lb", [COUT], F32, kind="ExternalInput").ap()
    embT_ap = nc.dram_tensor("embT", [COUT, K], F32, kind="ExternalInput").ap()
    eb2_ap = nc.dram_tensor("eb2", [K], F32, kind="ExternalInput").ap()  # 0.5*|e_k|^2
    idx_ap = nc.dram_tensor("idx", [NPOS], I32, kind="ExternalOutput").ap()

    with tile.TileContext(nc) as tc:
        for _ in range(n_iters):
            _emit_body(nc, tc, lat_ap, wt_ap, cb_ap, lw_ap, lb_ap, embT_ap,
                       eb2_ap, idx_ap)
    nc.compile()
    return nc


def _emit_body(nc, tc, lat_ap, wt_ap, cb_ap, lw_ap, lb_ap, embT_ap, eb2_ap,
               idx_ap):
    from contextlib import ExitStack
    ctx = ExitStack()
    with ctx:
        glob = ctx.enter_context(tc.tile_pool(name="glob", bufs=1))
        psp = ctx.enter_context(tc.tile_pool(name="ps", bufs=8, space="PSUM"))

        # ---- whole-kernel-lifetime tiles ----
        embT = [glob.tile([128, K], F32, name=f"embT{d}", tag=f"embT{d}")
                for d in range(2)]
        for d in range(2):
            nc.sync.dma_start(embT[d][:], embT_ap[d * 128:(d + 1) * 128, :])
        lwT = [glob.tile([128, COUT], F32, name=f"lwT{c}", tag=f"lwT{c}")
               for c in range(2)]
        for c in range(2):
            nc.sync.dma_start(lwT[c][:], lw_ap[c * 128:(c + 1) * 128, :])
        cbt = glob.tile([128, 2], F32, name="cbt")
        nc.sync.dma_start(cbt[:], cb_ap.rearrange("(c p) -> p c", p=128))
        lbt = glob.tile([128, 2], F32, name="lbt")
        nc.sync.dma_start(lbt[:], lb_ap.rearrange("(c p) -> p c", p=128))

        conv_sb = [glob.tile([128, NPOS], F32, name=f"conv_sb{c}", tag=f"conv_sb{c}")
                   for c in range(2)]
        flatT = [glob.tile([128, NPOS], F32, name=f"flatT{d}", tag=f"flatT{d}")
                 for d in range(2)]
        idx_all = glob.tile([128, NPC], U32, name="idx_all")

        # ---- phase 1: conv (9 shifted matmuls, H padded to 10 in SBUF) ----
        with tc.tile_pool(name="p1", bufs=1) as p1:
            latp = [p1.tile([128, BL, 10, 8], F32, name=f"latp{c}", tag=f"latp{c}")
                    for c in range(4)]
            lat_cbhw = lat_ap.rearrange("b c h w -> c b h w")
            for c in range(4):
                nc.vector.memset(latp[c][:], 0.0)
                nc.sync.dma_start(latp[c][:, :, 1:9, :],
                                  lat_cbhw[c * 128:(c + 1) * 128])
            wts = [p1.tile([128, 9, COUT], F32, name=f"wts{c}", tag=f"wts{c}")
                   for c in range(4)]
            wt_r = wt_ap.rearrange("ky kx ci co -> ci (ky kx) co")
            for c in range(4):
                nc.sync.dma_start(wts[c][:], wt_r[c * 128:(c + 1) * 128])

            shifts = [(0, 0)] + [(dy, dx) for dy in (-1, 0, 1) for dx in (-1, 0, 1)
                                 if (dy, dx) != (0, 0)]
            for bc in range(4):          # 8 images -> 512 positions each
                for cc in range(2):      # cout chunk
                    ps = psp.tile([128, 512], F32, name="ps_conv", tag="ps")
                    ps_v = ps[:].rearrange("p (b h w) -> p (b h) w", h=8, w=8)
                    first = True
                    for (dy, dx) in shifts:
                        w0, w1 = max(0, -dx), 8 - max(0, dx)
                        k = (dy + 1) * 3 + (dx + 1)
                        for ci in range(4):
                            rhs = latp[ci][:, bc * 8:(bc + 1) * 8, 1 + dy:9 + dy,
                                           w0 + dx:w1 + dx]
                            out = ps_v[:, :, w0:w1]
                            nc.tensor.matmul(
                                out, wts[ci][:, k, cc * 128:(cc + 1) * 128], rhs,
                                start=first,
                                stop=(dy == 1 and dx == 1 and ci == 3))
                            first = False
                    nc.scalar.add(conv_sb[cc][:, bc * 512:(bc + 1) * 512], ps[:],
                                  cbt[:, cc:cc + 1])

        # ---- phase 2: linear ----
        eb2b = glob.tile([128, K], F32, name="eb2b")
        nc.sync.dma_start(eb2b[:], eb2_ap[:].unsqueeze(0).partition_broadcast(128))
        for bc in range(4):
            for dc in range(2):      # dout chunk
                ps = psp.tile([128, 512], F32, name="ps_lin", tag="ps")
                for cc in range(2):
                    nc.tensor.matmul(
                        ps[:], lwT[cc][:, dc * 128:(dc + 1) * 128],
                        conv_sb[cc][:, bc * 512:(bc + 1) * 512],
                        start=(cc == 0), stop=(cc == 1))
                nc.scalar.add(flatT[dc][:, bc * 512:(bc + 1) * 512], ps[:],
                              lbt[:, dc:dc + 1])

        # ---- phase 3: scores + argmax per pos-chunk ----
        scp = ctx.enter_context(tc.tile_pool(name="scp", bufs=2))
        for pc in range(NPC):
            scores = scp.tile([128, K], F32, name="scores", tag="scores")
            for kh in range(2):
                pss = []
                for d in range(2):
                    for kc in range(8):
                        kci = kh * 8 + kc
                        if d == 0:
                            ps = psp.tile([128, 512], F32, name="ps_s", tag="ps")
                            pss.append(ps)
                        nc.tensor.matmul(
                            pss[kc][:], flatT[d][:, pc * 128:(pc + 1) * 128],
                            embT[d][:, kci * 512:(kci + 1) * 512],
                            start=(d == 0), stop=(d == 1))
                for kc in range(8):
                    kci = kh * 8 + kc
                    nc.vector.tensor_tensor(
                        out=scores[:, kci * 512:(kci + 1) * 512],
                        in0=pss[kc][:], in1=eb2b[:, kci * 512:(kci + 1) * 512],
                        op=mybir.AluOpType.subtract)
            m8 = scp.tile([128, 8], F32, name="m8", tag="m8")
            nc.vector.max(m8[:], scores[:])
            mi8 = scp.tile([128, 8], U32, name="mi8", tag="mi8")
            nc.vector.max_index(mi8[:], m8[:], scores[:])
            nc.vector.tensor_copy(idx_all[:, pc:pc + 1], mi8[:, 0:1])

        # ---- output ----
        nc.sync.dma_start(idx_ap.rearrange("(c r) -> r c", r=128),
                          idx_all[:].bitcast(I32))


def _get_nc(n_iters=1):
    key = n_iters
    if key not in _CACHE:
        _CACHE[key] = build_kernel(n_iters)
    return _CACHE[key]


def prepare_inputs(latent, conv_w, conv_b, lin_w, lin_b, emb):
    """Host-side input prep: layout transforms + per-core batch shards."""
    wt = np.ascontiguousarray(conv_w.transpose(2, 3, 1, 0))       # [ky,kx,cin,cout]
    lwT = np.ascontiguousarray(lin_w.T)                           # [cout_in, dout]
    embT = np.ascontiguousarray(emb.T)                            # [d, K]
    eb2 = (0.5 * np.sum(emb.astype(np.float64) ** 2, axis=1)).astype(np.float32)
    in_maps = []
    for c in range(NCORES):
        in_maps.append({
            "lat": np.ascontiguousarray(latent[c * BL:(c + 1) * BL]),
            "wt": wt, "cb": conv_b, "lw": lwT, "lb": lin_b,
            "embT": embT, "eb2": eb2,
        })
    return in_maps


def kernel(latent, conv_w, conv_b, lin_w, lin_b, emb):
    latent = np.asarray(latent, dtype=np.float32)
    conv_w = np.asarray(conv_w, dtype=np.float32)
    conv_b = np.asarray(conv_b, dtype=np.float32)
    lin_w = np.asarray(lin_w, dtype=np.float32)
    lin_b = np.asarray(lin_b, dtype=np.float32)
    emb = np.asarray(emb, dtype=np.float32)

    nc = _get_nc(1)
    in_maps = prepare_inputs(latent, conv_w, conv_b, lin_w, lin_b, emb)
    res = run_bass_kernel_spmd(nc, in_maps, core_ids=list(range(NCORES)))
    out = np.concatenate([res.results[c]["idx"] for c in range(NCORES)])
    return out.reshape(-1, 64).astype(np.int32)


# revision 4
# speedup vs baseline: 1.1538x; 1.1538x over previous
"""VQ codebook nearest-neighbor encode kernel for Trainium2 (8 NeuronCores).

Pipeline (matches the reference nn.Module):
  1. 3x3 SAME conv  [B,512,8,8] -> [B,256,8,8]   (9 shifted matmuls, PSUM accum)
  2. permute + linear on channel dim              (matmuls)
  3. nearest codeword: argmin_k |x - e_k|^2 = argmax_k (x.e_k - 0.5|e_k|^2)
     via matmuls + DVE subtract + DVE max8/max_index.

Precision: all matmuls run as fp16 hi/lo split 3-pass products
(a.b ~= ah.bh + ah.bl + al.bh, fp32 PSUM accumulation), which is ~2x faster
than native fp32 matmuls on the PE while keeping ~2^-24 relative error, so
the computed argmin matches the fp32 reference exactly in practice.

Biases are folded on the host: conv_b/lin_b only shift every position's x by
a constant vector c, so  x.e_k - 0.5|e_k|^2 = y.e_k - (0.5|e_k|^2 - c.e_k)
with y the bias-free linear output; the per-codeword constant is merged into
the eb2 table.

Sharding: data-parallel over batch. Each of the 8 cores processes 32 images
(2048 positions) end-to-end with a full copy of the (small) weights and the
codebook. No collectives; host concatenates the 8 index shards.
"""

import sys

sys.path.insert(0, "/opt/trn_rl_repo")

import numpy as np

import concourse.bass as bass  # noqa: F401  (registers engines)
import concourse.tile as tile
from concourse import bacc, mybir
from concourse.bass_utils import run_bass_kernel_spmd

F16 = mybir.dt.float16
F32 = mybir.dt.float32
U32 = mybir.dt.uint32
I32 = mybir.dt.int32

B, CIN, COUT, H, W, K = 256, 512, 256, 8, 8, 8192
NCORES = 8
BL = B // NCORES          # images per core (32)
NPOS = BL * H * W         # positions per core (2048)
NPC = NPOS // 128         # pos-chunks per core (16)
NKC = K // 512            # codebook chunks (16)

_CACHE = {}


def build_kernel(n_iters: int = 1):
    nc = bacc.Bacc("TRN2", target_bir_lowering=False, debug=False,
                   num_devices=NCORES)
    aps = {}
    for nm, shape, dt in [
        ("lat_h", [BL, CIN, H, W], F16), ("lat_l", [BL, CIN, H, W], F16),
        ("wt_h", [3, 3, CIN, COUT], F16), ("wt_l", [3, 3, CIN, COUT], F16),
        ("lw_h", [COUT, COUT], F16), ("lw_l", [COUT, COUT], F16),
        ("emb_h", [COUT, K], F16), ("emb_l", [COUT, K], F16),
        ("eb2", [K], F32),
    ]:
        aps[nm] = nc.dram_tensor(nm, shape, dt, kind="ExternalInput").ap()
    idx_ap = nc.dram_tensor("idx", [NPOS], I32, kind="ExternalOutput").ap()

    with tile.TileContext(nc) as tc:
        for _ in range(n_iters):
            _emit_body(nc, tc, aps, idx_ap)
    nc.compile()
    return nc


def _emit_body(nc, tc, aps, idx_ap):
    from contextlib import ExitStack
    ctx = ExitStack()
    with ctx:
        glob = ctx.enter_context(tc.tile_pool(name="glob", bufs=1))
        psp = ctx.enter_context(tc.tile_pool(name="ps", bufs=8, space="PSUM"))

        # ---- whole-kernel-lifetime tiles ----
        emb = {}
        for p in ("h", "l"):
            for d in range(2):
                t = glob.tile([128, K], F16, name=f"emb_{p}{d}", tag=f"emb_{p}{d}")
                nc.sync.dma_start(t[:], aps[f"emb_{p}"][d * 128:(d + 1) * 128, :])
                emb[p, d] = t
        lw = {}
        for p in ("h", "l"):
            for c in range(2):
                t = glob.tile([128, COUT], F16, name=f"lw_{p}{c}", tag=f"lw_{p}{c}")
                nc.sync.dma_start(t[:], aps[f"lw_{p}"][c * 128:(c + 1) * 128, :])
                lw[p, c] = t
        eb2b = glob.tile([128, K], F32, name="eb2b")
        nc.sync.dma_start(eb2b[:], aps["eb2"][:].unsqueeze(0).partition_broadcast(128))

        conv = {}   # (p, cout_chunk) -> [128, NPOS] fp16, bias-free conv out
        for p in ("h", "l"):
            for c in range(2):
                conv[p, c] = glob.tile([128, NPOS], F16, name=f"conv_{p}{c}",
                                       tag=f"conv_{p}{c}")
        flat = {}   # (p, dout_chunk) -> [128, NPOS] fp16, bias-free linear out
        for p in ("h", "l"):
            for d in range(2):
                flat[p, d] = glob.tile([128, NPOS], F16, name=f"flat_{p}{d}",
                                       tag=f"flat_{p}{d}")
        idx_all = glob.tile([128, NPC], U32, name="idx_all")

        # ---- phase 1: conv (9 shifted matmuls, H padded to 10 in SBUF) ----
        with tc.tile_pool(name="p1", bufs=1) as p1:
            latp = {}
            for p in ("h", "l"):
                src = aps[f"lat_{p}"].rearrange("b c h w -> c b h w")
                for c in range(4):
                    t = p1.tile([128, BL, 10, 8], F16, name=f"latp_{p}{c}",
                                tag=f"latp_{p}{c}")
                    nc.vector.memset(t[:], 0.0)
                    nc.sync.dma_start(t[:, :, 1:9, :], src[c * 128:(c + 1) * 128])
                    latp[p, c] = t
            wts = {}
            for p in ("h", "l"):
                src = aps[f"wt_{p}"].rearrange("ky kx ci co -> ci (ky kx) co")
                for c in range(4):
                    t = p1.tile([128, 9, COUT], F16, name=f"wts_{p}{c}",
                                tag=f"wts_{p}{c}")
                    nc.sync.dma_start(t[:], src[c * 128:(c + 1) * 128])
                    wts[p, c] = t

            shifts = [(0, 0)] + [(dy, dx) for dy in (-1, 0, 1) for dx in (-1, 0, 1)
                                 if (dy, dx) != (0, 0)]
            prods = [("h", "h"), ("h", "l"), ("l", "h")]  # (weight, latent)
            for bc in range(4):          # 8 images -> 512 positions each
                for cc in range(2):      # cout chunk
                    ps = psp.tile([128, 512], F32, name="ps_conv", tag="ps")
                    ps_v = ps[:].rearrange("p (b h w) -> p (b h) w", h=8, w=8)
                    n_mm = len(shifts) * 4 * len(prods)
                    i_mm = 0
                    for (dy, dx) in shifts:
                        w0, w1 = max(0, -dx), 8 - max(0, dx)
                        k = (dy + 1) * 3 + (dx + 1)
                        for ci in range(4):
                            for (pw, pl) in prods:
                                rhs = latp[pl, ci][:, bc * 8:(bc + 1) * 8,
                                                   1 + dy:9 + dy,
                                                   w0 + dx:w1 + dx]
                                nc.tensor.matmul(
                                    ps_v[:, :, w0:w1],
                                    wts[pw, ci][:, k, cc * 128:(cc + 1) * 128],
                                    rhs, start=(i_mm == 0),
                                    stop=(i_mm == n_mm - 1))
                                i_mm += 1
                    sl = slice(bc * 512, (bc + 1) * 512)
                    nc.scalar.copy(conv["h", cc][:, sl], ps[:])
                    nc.vector.tensor_tensor(out=conv["l", cc][:, sl], in0=ps[:],
                                            in1=conv["h", cc][:, sl],
                                            op=mybir.AluOpType.subtract)

        # ---- phase 2: linear (bias-free) ----
        for bc in range(4):
            sl = slice(bc * 512, (bc + 1) * 512)
            for dc in range(2):      # dout chunk
                ps = psp.tile([128, 512], F32, name="ps_lin", tag="ps")
                n_mm = 2 * 3
                i_mm = 0
                for cc in range(2):
                    for (pw, px) in (("h", "h"), ("h", "l"), ("l", "h")):
                        nc.tensor.matmul(
                            ps[:], lw[pw, cc][:, dc * 128:(dc + 1) * 128],
                            conv[px, cc][:, sl],
                            start=(i_mm == 0), stop=(i_mm == n_mm - 1))
                        i_mm += 1
                nc.scalar.copy(flat["h", dc][:, sl], ps[:])
                nc.vector.tensor_tensor(out=flat["l", dc][:, sl], in0=ps[:],
                                        in1=flat["h", dc][:, sl],
                                        op=mybir.AluOpType.subtract)

        # ---- phase 3: scores + argmax per pos-chunk ----
        scp = ctx.enter_context(tc.tile_pool(name="scp", bufs=2))
        passes = [("h", "h"), ("h", "l"), ("l", "h")]  # (flat, emb)
        for pc in range(NPC):
            psl = slice(pc * 128, (pc + 1) * 128)
            scores = scp.tile([128, K], F32, name="scores", tag="scores")
            for kh in range(2):
                pss = [psp.tile([128, 512], F32, name="ps_s", tag="ps")
                       for _ in range(8)]
                for ip, (pf, pe) in enumerate(passes):
                    for d in range(2):
                        for kc in range(8):
                            kci = kh * 8 + kc
                            nc.tensor.matmul(
                                pss[kc][:], flat[pf, d][:, psl],
                                emb[pe, d][:, kci * 512:(kci + 1) * 512],
                                start=(ip == 0 and d == 0),
                                stop=(ip == len(passes) - 1 and d == 1))
                for kc in range(8):
                    kci = kh * 8 + kc
                    nc.vector.tensor_tensor(
                        out=scores[:, kci * 512:(kci + 1) * 512],
                        in0=pss[kc][:], in1=eb2b[:, kci * 512:(kci + 1) * 512],
                        op=mybir.AluOpType.subtract)
            m8 = scp.tile([128, 8], F32, name="m8", tag="m8")
            nc.vector.max(m8[:], scores[:])
            mi8 = scp.tile([128, 8], U32, name="mi8", tag="mi8")
            nc.vector.max_index(mi8[:], m8[:], scores[:])
            nc.vector.tensor_copy(idx_all[:, pc:pc + 1], mi8[:, 0:1])

        # ---- output ----
        nc.sync.dma_start(idx_ap.rearrange("(c r) -> r c", r=128),
                          idx_all[:].bitcast(I32))


def _get_nc(n_iters=1):
    key = n_iters
    if key not in _CACHE:
        _CACHE[key] = build_kernel(n_iters)
    return _CACHE[key]


def _split16(a):
    h = a.astype(np.float16)
    l = (a.astype(np.float32) - h.astype(np.float32)).astype(np.float16)
    return h, l


def prepare_inputs(latent, conv_w, conv_b, lin_w, lin_b, emb):
    """Host-side input prep: layout transforms, fp16 hi/lo splits, shards."""
    wt = np.ascontiguousarray(conv_w.transpose(2, 3, 1, 0))       # [ky,kx,cin,cout]
    wt_h, wt_l = _split16(wt)
    lwT = np.ascontiguousarray(lin_w.T)                           # [cout_in, dout]
    lw_h, lw_l = _split16(lwT)
    embT = np.ascontiguousarray(emb.T)                            # [d, K]
    emb_h, emb_l = _split16(embT)
    # fold conv/linear biases into the per-codeword constant:
    # x = y + c with c = lin_w @ conv_b + lin_b, so
    # argmin_k |x-e_k|^2 = argmax_k ( y.e_k - (0.5|e_k|^2 - c.e_k) )
    c = lin_w.astype(np.float64) @ conv_b.astype(np.float64) + lin_b.astype(np.float64)
    eb2 = (0.5 * np.sum(emb.astype(np.float64) ** 2, axis=1)
           - emb.astype(np.float64) @ c).astype(np.float32)
    in_maps = []
    for ci in range(NCORES):
        lat_h, lat_l = _split16(latent[ci * BL:(ci + 1) * BL])
        in_maps.append({
            "lat_h": lat_h, "lat_l": lat_l,
            "wt_h": wt_h, "wt_l": wt_l, "lw_h": lw_h, "lw_l": lw_l,
            "emb_h": emb_h, "emb_l": emb_l, "eb2": eb2,
        })
    return in_maps


def kernel(latent, conv_w, conv_b, lin_w, lin_b, emb):
    latent = np.asarray(latent, dtype=np.float32)
    conv_w = np.asarray(conv_w, dtype=np.float32)
    conv_b = np.asarray(conv_b, dtype=np.float32)
    lin_w = np.asarray(lin_w, dtype=np.float32)
    lin_b = np.asarray(lin_b, dtype=np.float32)
    emb = np.asarray(emb, dtype=np.float32)

    nc = _get_nc(1)
    in_maps = prepare_inputs(latent, conv_w, conv_b, lin_w, lin_b, emb)
    res = run_bass_kernel_spmd(nc, in_maps, core_ids=list(range(NCORES)))
    out = np.concatenate([res.results[c]["idx"] for c in range(NCORES)])
    return out.reshape(-1, 64).astype(np.int32)


# revision 5
# speedup vs baseline: 1.3301x; 1.1528x over previous
"""VQ codebook nearest-neighbor encode kernel for Trainium2 (8 NeuronCores).

Pipeline (matches the reference nn.Module):
  1. 3x3 SAME conv  [B,512,8,8] -> [B,256,8,8]   (9 shifted matmuls, PSUM accum)
  2. permute + linear on channel dim              (matmuls)
  3. nearest codeword: argmin_k |x - e_k|^2 = argmax_k (x.e_k - 0.5|e_k|^2)
     via matmuls + DVE subtract + DVE max8/max_index.

Precision: all matmuls run as fp16 hi/lo split 3-pass products
(a.b ~= ah.bh + ah.bl + al.bh, fp32 PSUM accumulation), which is ~2x faster
than native fp32 matmuls on the PE while keeping ~2^-24 relative error, so
the computed argmin matches the fp32 reference exactly in practice.

Biases are folded on the host: conv_b/lin_b only shift every position's x by
a constant vector c, so  x.e_k - 0.5|e_k|^2 = y.e_k - (0.5|e_k|^2 - c.e_k)
with y the bias-free linear output; the per-codeword constant is merged into
the eb2 table.

Sharding: data-parallel over batch. Each of the 8 cores processes 32 images
(2048 positions) end-to-end with a full copy of the (small) weights and the
codebook. No collectives; host concatenates the 8 index shards.
"""

import sys

sys.path.insert(0, "/opt/trn_rl_repo")

import numpy as np

import concourse.bass as bass  # noqa: F401  (registers engines)
import concourse.tile as tile
from concourse import bacc, mybir
from concourse.bass_utils import run_bass_kernel_spmd

F16 = mybir.dt.float16
F32 = mybir.dt.float32
U32 = mybir.dt.uint32
I32 = mybir.dt.int32

B, CIN, COUT, H, W, K = 256, 512, 256, 8, 8, 8192
NCORES = 8
BL = B // NCORES          # images per core (32)
NPOS = BL * H * W         # positions per core (2048)
NPC = NPOS // 128         # pos-chunks per core (16)
NKC = K // 512            # codebook chunks (16)

_CACHE = {}


def build_kernel(n_iters: int = 1):
    nc = bacc.Bacc("TRN2", target_bir_lowering=False, debug=False,
                   num_devices=NCORES)
    aps = {}
    for nm, shape, dt in [
        ("lat_h", [BL, CIN, H, W], F16), ("lat_l", [BL, CIN, H, W], F16),
        ("wt_h", [3, 3, CIN, COUT], F16), ("wt_l", [3, 3, CIN, COUT], F16),
        ("lw_h", [COUT, COUT], F16), ("lw_l", [COUT, COUT], F16),
        ("emb_h", [COUT, K], F16), ("emb_l", [COUT, K], F16),
        ("eb2", [K], F32),
    ]:
        aps[nm] = nc.dram_tensor(nm, shape, dt, kind="ExternalInput").ap()
    idx_ap = nc.dram_tensor("idx", [NPOS], I32, kind="ExternalOutput").ap()

    with tile.TileContext(nc) as tc:
        for _ in range(n_iters):
            _emit_body(nc, tc, aps, idx_ap)
    nc.compile()
    return nc


def _emit_body(nc, tc, aps, idx_ap):
    from contextlib import ExitStack
    ctx = ExitStack()
    with ctx:
        glob = ctx.enter_context(tc.tile_pool(name="glob", bufs=1))
        psp = ctx.enter_context(tc.tile_pool(name="ps", bufs=8, space="PSUM"))

        # ---- whole-kernel-lifetime tiles ----
        emb = {}
        for p in ("h", "l"):
            for d in range(2):
                t = glob.tile([128, K], F16, name=f"emb_{p}{d}", tag=f"emb_{p}{d}")
                nc.sync.dma_start(t[:], aps[f"emb_{p}"][d * 128:(d + 1) * 128, :])
                emb[p, d] = t
        lw = {}
        for p in ("h", "l"):
            for c in range(2):
                t = glob.tile([128, COUT], F16, name=f"lw_{p}{c}", tag=f"lw_{p}{c}")
                nc.sync.dma_start(t[:], aps[f"lw_{p}"][c * 128:(c + 1) * 128, :])
                lw[p, c] = t
        eb2b = glob.tile([128, K], F32, name="eb2b")
        nc.sync.dma_start(eb2b[:], aps["eb2"][:].unsqueeze(0).partition_broadcast(128))

        conv = {}   # (p, cout_chunk) -> [128, NPOS] fp16, bias-free conv out
        for p in ("h", "l"):
            for c in range(2):
                conv[p, c] = glob.tile([128, NPOS], F16, name=f"conv_{p}{c}",
                                       tag=f"conv_{p}{c}")
        flat = {}   # (p, dout_chunk) -> [128, NPOS] fp16, bias-free linear out
        for p in ("h", "l"):
            for d in range(2):
                flat[p, d] = glob.tile([128, NPOS], F16, name=f"flat_{p}{d}",
                                       tag=f"flat_{p}{d}")
        idx_all = glob.tile([128, NPC], U32, name="idx_all")

        # ---- phase 1: conv (9 shifted matmuls, H padded to 10 in SBUF) ----
        with tc.tile_pool(name="p1", bufs=1) as p1:
            latp = {}
            for p in ("h", "l"):
                src = aps[f"lat_{p}"].rearrange("b c h w -> c b h w")
                for c in range(4):
                    t = p1.tile([128, BL, 10, 8], F16, name=f"latp_{p}{c}",
                                tag=f"latp_{p}{c}")
                    nc.vector.memset(t[:], 0.0)
                    nc.sync.dma_start(t[:, :, 1:9, :], src[c * 128:(c + 1) * 128])
                    latp[p, c] = t
            wts = {}
            for p in ("h", "l"):
                src = aps[f"wt_{p}"].rearrange("ky kx ci co -> ci (ky kx) co")
                for c in range(4):
                    t = p1.tile([128, 9, COUT], F16, name=f"wts_{p}{c}",
                                tag=f"wts_{p}{c}")
                    nc.sync.dma_start(t[:], src[c * 128:(c + 1) * 128])
                    wts[p, c] = t

            shifts = [(0, 0)] + [(dy, dx) for dy in (-1, 0, 1) for dx in (-1, 0, 1)
                                 if (dy, dx) != (0, 0)]
            prods = [("h", "h"), ("h", "l"), ("l", "h")]  # (weight, latent)
            for bc in range(4):          # 8 images -> 512 positions each
                for cc in range(2):      # cout chunk
                    ps = psp.tile([128, 512], F32, name="ps_conv", tag="ps")
                    ps_v = ps[:].rearrange("p (b h w) -> p (b h) w", h=8, w=8)
                    n_mm = len(shifts) * 4 * len(prods)
                    i_mm = 0
                    for (dy, dx) in shifts:
                        w0, w1 = max(0, -dx), 8 - max(0, dx)
                        k = (dy + 1) * 3 + (dx + 1)
                        for ci in range(4):
                            for (pw, pl) in prods:
                                rhs = latp[pl, ci][:, bc * 8:(bc + 1) * 8,
                                                   1 + dy:9 + dy,
                                                   w0 + dx:w1 + dx]
                                nc.tensor.matmul(
                                    ps_v[:, :, w0:w1],
                                    wts[pw, ci][:, k, cc * 128:(cc + 1) * 128],
                                    rhs, start=(i_mm == 0),
                                    stop=(i_mm == n_mm - 1))
                                i_mm += 1
                    sl = slice(bc * 512, (bc + 1) * 512)
                    nc.scalar.copy(conv["h", cc][:, sl], ps[:])
                    nc.vector.tensor_tensor(out=conv["l", cc][:, sl], in0=ps[:],
                                            in1=conv["h", cc][:, sl],
                                            op=mybir.AluOpType.subtract)

        # ---- phase 2: linear (bias-free) ----
        for bc in range(4):
            sl = slice(bc * 512, (bc + 1) * 512)
            for dc in range(2):      # dout chunk
                ps = psp.tile([128, 512], F32, name="ps_lin", tag="ps")
                n_mm = 2 * 3
                i_mm = 0
                for cc in range(2):
                    for (pw, px) in (("h", "h"), ("h", "l"), ("l", "h")):
                        nc.tensor.matmul(
                            ps[:], lw[pw, cc][:, dc * 128:(dc + 1) * 128],
                            conv[px, cc][:, sl],
                            start=(i_mm == 0), stop=(i_mm == n_mm - 1))
                        i_mm += 1
                nc.scalar.copy(flat["h", dc][:, sl], ps[:])
                nc.vector.tensor_tensor(out=flat["l", dc][:, sl], in0=ps[:],
                                        in1=flat["h", dc][:, sl],
                                        op=mybir.AluOpType.subtract)

        # ---- phase 3: scores + argmax per pos-chunk ----
        scp = ctx.enter_context(tc.tile_pool(name="scp", bufs=2))
        passes = [("h", "h"), ("h", "l"), ("l", "h")]  # (flat, emb)
        for pc in range(NPC):
            psl = slice(pc * 128, (pc + 1) * 128)
            scores = scp.tile([128, K], F32, name="scores", tag="scores")
            for kci in range(NKC):
                ps = psp.tile([128, 512], F32, name="ps_s", tag="ps")
                for ip, (pf, pe) in enumerate(passes):
                    for d in range(2):
                        nc.tensor.matmul(
                            ps[:], flat[pf, d][:, psl],
                            emb[pe, d][:, kci * 512:(kci + 1) * 512],
                            start=(ip == 0 and d == 0),
                            stop=(ip == len(passes) - 1 and d == 1))
                nc.vector.tensor_tensor(
                    out=scores[:, kci * 512:(kci + 1) * 512],
                    in0=ps[:], in1=eb2b[:, kci * 512:(kci + 1) * 512],
                    op=mybir.AluOpType.subtract)
            m8 = scp.tile([128, 8], F32, name="m8", tag="m8")
            nc.vector.max(m8[:], scores[:])
            mi8 = scp.tile([128, 8], U32, name="mi8", tag="mi8")
            nc.vector.max_index(mi8[:], m8[:], scores[:])
            nc.vector.tensor_copy(idx_all[:, pc:pc + 1], mi8[:, 0:1])

        # ---- output ----
        nc.sync.dma_start(idx_ap.rearrange("(c r) -> r c", r=128),
                          idx_all[:].bitcast(I32))


def _get_nc(n_iters=1):
    key = n_iters
    if key not in _CACHE:
        _CACHE[key] = build_kernel(n_iters)
    return _CACHE[key]


def _split16(a):
    h = a.astype(np.float16)
    l = (a.astype(np.float32) - h.astype(np.float32)).astype(np.float16)
    return h, l


def prepare_inputs(latent, conv_w, conv_b, lin_w, lin_b, emb):
    """Host-side input prep: layout transforms, fp16 hi/lo splits, shards."""
    wt = np.ascontiguousarray(conv_w.transpose(2, 3, 1, 0))       # [ky,kx,cin,cout]
    wt_h, wt_l = _split16(wt)
    lwT = np.ascontiguousarray(lin_w.T)                           # [cout_in, dout]
    lw_h, lw_l = _split16(lwT)
    embT = np.ascontiguousarray(emb.T)                            # [d, K]
    emb_h, emb_l = _split16(embT)
    # fold conv/linear biases into the per-codeword constant:
    # x = y + c with c = lin_w @ conv_b + lin_b, so
    # argmin_k |x-e_k|^2 = argmax_k ( y.e_k - (0.5|e_k|^2 - c.e_k) )
    c = lin_w.astype(np.float64) @ conv_b.astype(np.float64) + lin_b.astype(np.float64)
    eb2 = (0.5 * np.sum(emb.astype(np.float64) ** 2, axis=1)
           - emb.astype(np.float64) @ c).astype(np.float32)
    in_maps = []
    for ci in range(NCORES):
        lat_h, lat_l = _split16(latent[ci * BL:(ci + 1) * BL])
        in_maps.append({
            "lat_h": lat_h, "lat_l": lat_l,
            "wt_h": wt_h, "wt_l": wt_l, "lw_h": lw_h, "lw_l": lw_l,
            "emb_h": emb_h, "emb_l": emb_l, "eb2": eb2,
        })
    return in_maps


def kernel(latent, conv_w, conv_b, lin_w, lin_b, emb):
    latent = np.asarray(latent, dtype=np.float32)
    conv_w = np.asarray(conv_w, dtype=np.float32)
    conv_b = np.asarray(conv_b, dtype=np.float32)
    lin_w = np.asarray(lin_w, dtype=np.float32)
    lin_b = np.asarray(lin_b, dtype=np.float32)
    emb = np.asarray(emb, dtype=np.float32)

    nc = _get_nc(1)
    in_maps = prepare_inputs(latent, conv_w, conv_b, lin_w, lin_b, emb)
    res = run_bass_kernel_spmd(nc, in_maps, core_ids=list(range(NCORES)))
    out = np.concatenate([res.results[c]["idx"] for c in range(NCORES)])
    return out.reshape(-1, 64).astype(np.int32)


# revision 7
# speedup vs baseline: 2.1434x; 1.6114x over previous
"""VQ codebook nearest-neighbor encode kernel for Trainium2 (8 NeuronCores).

Pipeline (matches the reference nn.Module):
  1. 3x3 SAME conv  [B,512,8,8] -> [B,256,8,8]   (9 shifted matmuls, PSUM accum)
  2. permute + linear on channel dim              (matmuls)
  3. nearest codeword: argmin_k |x - e_k|^2 = argmax_k (x.e_k - 0.5|e_k|^2)
     via matmuls + DVE subtract + DVE max8/max_index.

Precision: all matmuls run as fp16 hi/lo split 3-pass products
(a.b ~= ah.bh + ah.bl + al.bh, fp32 PSUM accumulation), which is ~2x faster
than native fp32 matmuls on the PE while keeping ~2^-24 relative error, so
the computed argmin matches the fp32 reference exactly in practice.

Biases are folded on the host: conv_b/lin_b only shift every position's x by
a constant vector c, so  x.e_k - 0.5|e_k|^2 = y.e_k - (0.5|e_k|^2 - c.e_k)
with y the bias-free linear output; the per-codeword constant is merged into
the eb2 table.

Sharding: data-parallel over batch. Each of the 8 cores processes 32 images
(2048 positions) end-to-end with a full copy of the (small) weights and the
codebook. No collectives; host concatenates the 8 index shards.
"""

import sys

sys.path.insert(0, "/opt/trn_rl_repo")

import numpy as np

import concourse.bass as bass  # noqa: F401  (registers engines)
import concourse.tile as tile
from concourse import bacc, mybir
from concourse.bass_utils import run_bass_kernel_spmd

F16 = mybir.dt.float16
F32 = mybir.dt.float32
U32 = mybir.dt.uint32
I32 = mybir.dt.int32

B, CIN, COUT, H, W, K = 256, 512, 256, 8, 8, 8192
NCORES = 8
BL = B // NCORES          # images per core (32)
NPOS = BL * H * W         # positions per core (2048)
NPC = NPOS // 128         # pos-chunks per core (16)
NKC = K // 512            # codebook chunks (16)

_CACHE = {}


def build_kernel(n_iters: int = 1):
    nc = bacc.Bacc("TRN2", target_bir_lowering=False, debug=False,
                   num_devices=NCORES)
    aps = {}
    for nm, shape, dt in [
        ("lat_h", [BL, CIN, H, W], F16), ("lat_l", [BL, CIN, H, W], F16),
        ("wt_h", [3, 3, CIN, COUT], F16), ("wt_l", [3, 3, CIN, COUT], F16),
        ("lw_h", [COUT, COUT], F16), ("lw_l", [COUT, COUT], F16),
        ("emb_h", [COUT, K], F16), ("emb_l", [COUT, K], F16),
        ("eb2", [K], F32),
    ]:
        aps[nm] = nc.dram_tensor(nm, shape, dt, kind="ExternalInput").ap()
    idx_ap = nc.dram_tensor("idx", [NPOS], I32, kind="ExternalOutput").ap()

    with tile.TileContext(nc) as tc:
        for _ in range(n_iters):
            _emit_body(nc, tc, aps, idx_ap)
    nc.compile()
    return nc


def _emit_body(nc, tc, aps, idx_ap):
    from contextlib import ExitStack
    ctx = ExitStack()
    with ctx:
        glob = ctx.enter_context(tc.tile_pool(name="glob", bufs=1))
        psp = ctx.enter_context(tc.tile_pool(name="ps", bufs=8, space="PSUM"))

        # ---- whole-kernel-lifetime tiles ----
        emb = {}
        for p in ("h", "l"):
            for d in range(2):
                t = glob.tile([128, K], F16, name=f"emb_{p}{d}", tag=f"emb_{p}{d}")
                nc.sync.dma_start(t[:], aps[f"emb_{p}"][d * 128:(d + 1) * 128, :])
                emb[p, d] = t
        lw = {}
        for p in ("h", "l"):
            for c in range(2):
                t = glob.tile([128, COUT], F16, name=f"lw_{p}{c}", tag=f"lw_{p}{c}")
                nc.sync.dma_start(t[:], aps[f"lw_{p}"][c * 128:(c + 1) * 128, :])
                lw[p, c] = t
        eb2b = glob.tile([128, K], F32, name="eb2b")
        nc.sync.dma_start(eb2b[:], aps["eb2"][:].unsqueeze(0).partition_broadcast(128))

        conv = {}   # (p, cout_chunk) -> [128, NPOS] fp16, bias-free conv out
        for p in ("h", "l"):
            for c in range(2):
                conv[p, c] = glob.tile([128, NPOS], F16, name=f"conv_{p}{c}",
                                       tag=f"conv_{p}{c}")
        flat = {}   # (p, dout_chunk) -> [128, NPOS] fp16, bias-free linear out
        for p in ("h", "l"):
            for d in range(2):
                flat[p, d] = glob.tile([128, NPOS], F16, name=f"flat_{p}{d}",
                                       tag=f"flat_{p}{d}")
        idx_all = glob.tile([128, NPC], U32, name="idx_all")

        # ---- phase 1: conv (9 shifted matmuls, H padded to 10 in SBUF) ----
        with tc.tile_pool(name="p1", bufs=1) as p1:
            latp = {}
            for p in ("h", "l"):
                src = aps[f"lat_{p}"].rearrange("b c h w -> c b h w")
                for c in range(4):
                    t = p1.tile([128, BL, 10, 8], F16, name=f"latp_{p}{c}",
                                tag=f"latp_{p}{c}")
                    nc.vector.memset(t[:], 0.0)
                    nc.sync.dma_start(t[:, :, 1:9, :], src[c * 128:(c + 1) * 128])
                    latp[p, c] = t
            wts = {}
            for p in ("h", "l"):
                src = aps[f"wt_{p}"].rearrange("ky kx ci co -> ci (ky kx) co")
                for c in range(4):
                    t = p1.tile([128, 9, COUT], F16, name=f"wts_{p}{c}",
                                tag=f"wts_{p}{c}")
                    nc.sync.dma_start(t[:], src[c * 128:(c + 1) * 128])
                    wts[p, c] = t

            shifts = [(0, 0)] + [(dy, dx) for dy in (-1, 0, 1) for dx in (-1, 0, 1)
                                 if (dy, dx) != (0, 0)]
            prods = [("h", "h"), ("h", "l"), ("l", "h")]  # (weight, latent)
            for bc in range(4):          # 8 images -> 512 positions each
                for cc in range(2):      # cout chunk
                    ps = psp.tile([128, 512], F32, name="ps_conv", tag="ps")
                    ps_v = ps[:].rearrange("p (b h w) -> p (b h) w", h=8, w=8)
                    n_mm = len(shifts) * 4 * len(prods)
                    i_mm = 0
                    for (dy, dx) in shifts:
                        w0, w1 = max(0, -dx), 8 - max(0, dx)
                        k = (dy + 1) * 3 + (dx + 1)
                        for ci in range(4):
                            for (pw, pl) in prods:
                                rhs = latp[pl, ci][:, bc * 8:(bc + 1) * 8,
                                                   1 + dy:9 + dy,
                                                   w0 + dx:w1 + dx]
                                nc.tensor.matmul(
                                    ps_v[:, :, w0:w1],
                                    wts[pw, ci][:, k, cc * 128:(cc + 1) * 128],
                                    rhs, start=(i_mm == 0),
                                    stop=(i_mm == n_mm - 1))
                                i_mm += 1
                    sl = slice(bc * 512, (bc + 1) * 512)
                    nc.scalar.copy(conv["h", cc][:, sl], ps[:])
                    nc.vector.tensor_tensor(out=conv["l", cc][:, sl], in0=ps[:],
                                            in1=conv["h", cc][:, sl],
                                            op=mybir.AluOpType.subtract)

        # ---- phase 2: linear (bias-free) ----
        for bc in range(4):
            sl = slice(bc * 512, (bc + 1) * 512)
            for dc in range(2):      # dout chunk
                ps = psp.tile([128, 512], F32, name="ps_lin", tag="ps")
                n_mm = 2 * 3
                i_mm = 0
                for cc in range(2):
                    for (pw, px) in (("h", "h"), ("h", "l"), ("l", "h")):
                        nc.tensor.matmul(
                            ps[:], lw[pw, cc][:, dc * 128:(dc + 1) * 128],
                            conv[px, cc][:, sl],
                            start=(i_mm == 0), stop=(i_mm == n_mm - 1))
                        i_mm += 1
                nc.scalar.copy(flat["h", dc][:, sl], ps[:])
                nc.vector.tensor_tensor(out=flat["l", dc][:, sl], in0=ps[:],
                                        in1=flat["h", dc][:, sl],
                                        op=mybir.AluOpType.subtract)

        # ---- phase 3: scores + argmax per pos-chunk ----
        scp = ctx.enter_context(tc.tile_pool(name="scp", bufs=2))
        passes = [("h", "h"), ("h", "l"), ("l", "h")]  # (flat, emb)
        for pc in range(NPC):
            psl = slice(pc * 128, (pc + 1) * 128)
            scores = scp.tile([128, K], F32, name="scores", tag="scores")
            for kci in range(NKC):
                ps = psp.tile([128, 512], F32, name="ps_s", tag="ps")
                for ip, (pf, pe) in enumerate(passes):
                    for d in range(2):
                        nc.tensor.matmul(
                            ps[:], flat[pf, d][:, psl],
                            emb[pe, d][:, kci * 512:(kci + 1) * 512],
                            start=(ip == 0 and d == 0),
                            stop=(ip == len(passes) - 1 and d == 1))
                nc.vector.tensor_tensor(
                    out=scores[:, kci * 512:(kci + 1) * 512],
                    in0=ps[:], in1=eb2b[:, kci * 512:(kci + 1) * 512],
                    op=mybir.AluOpType.subtract)
            m8 = scp.tile([128, 8], F32, name="m8", tag="m8")
            nc.vector.max(m8[:], scores[:])
            mi8 = scp.tile([128, 8], U32, name="mi8", tag="mi8")
            nc.vector.max_index(mi8[:], m8[:], scores[:])
            nc.vector.tensor_copy(idx_all[:, pc:pc + 1], mi8[:, 0:1])

        # ---- output ----
        nc.sync.dma_start(idx_ap.rearrange("(c r) -> r c", r=128),
                          idx_all[:].bitcast(I32))


def _get_nc(n_iters=1):
    key = n_iters
    if key not in _CACHE:
        _CACHE[key] = build_kernel(n_iters)
    return _CACHE[key]


def _split16(a):
    h = a.astype(np.float16)
    l = (a.astype(np.float32) - h.astype(np.float32)).astype(np.float16)
    return h, l


def prepare_inputs(latent, conv_w, conv_b, lin_w, lin_b, emb):
    """Host-side input prep: layout transforms, fp16 hi/lo splits, shards."""
    wt = np.ascontiguousarray(conv_w.transpose(2, 3, 1, 0))       # [ky,kx,cin,cout]
    wt_h, wt_l = _split16(wt)
    lwT = np.ascontiguousarray(lin_w.T)                           # [cout_in, dout]
    lw_h, lw_l = _split16(lwT)
    embT = np.ascontiguousarray(emb.T)                            # [d, K]
    emb_h, emb_l = _split16(embT)
    # fold conv/linear biases into the per-codeword constant:
    # x = y + c with c = lin_w @ conv_b + lin_b, so
    # argmin_k |x-e_k|^2 = argmax_k ( y.e_k - (0.5|e_k|^2 - c.e_k) )
    c = lin_w.astype(np.float64) @ conv_b.astype(np.float64) + lin_b.astype(np.float64)
    eb2 = (0.5 * np.sum(emb.astype(np.float64) ** 2, axis=1)
           - emb.astype(np.float64) @ c).astype(np.float32)
    in_maps = []
    for ci in range(NCORES):
        lat_h, lat_l = _split16(latent[ci * BL:(ci + 1) * BL])
        in_maps.append({
            "lat_h": lat_h, "lat_l": lat_l,
            "wt_h": wt_h, "wt_l": wt_l, "lw_h": lw_h, "lw_l": lw_l,
            "emb_h": emb_h, "emb_l": emb_l, "eb2": eb2,
        })
    return in_maps


def kernel(latent, conv_w, conv_b, lin_w, lin_b, emb):
    latent = np.asarray(latent, dtype=np.float32)
    conv_w = np.asarray(conv_w, dtype=np.float32)
    conv_b = np.asarray(conv_b, dtype=np.float32)
    lin_w = np.asarray(lin_w, dtype=np.float32)
    lin_b = np.asarray(lin_b, dtype=np.float32)
    emb = np.asarray(emb, dtype=np.float32)

    nc = _get_nc(1)
    in_maps = prepare_inputs(latent, conv_w, conv_b, lin_w, lin_b, emb)
    res = run_bass_kernel_spmd(nc, in_maps, core_ids=list(range(NCORES)))
    out = np.concatenate([res.results[c]["idx"] for c in range(NCORES)])
    return out.reshape(-1, 64).astype(np.int32)
